# revision 3
# baseline (speedup 1.0000x reference)
"""Depthwise causal conv1d (K=4, dilation=1) on 8 TRN2 NeuronCores.

Reference: x [B=8, T=4096, C=1024] f32, W [4, 1, 1024] f32 (WIO layout),
y[b, t, c] = sum_k W[k, 0, c] * x[b, t - 3 + k, c]  (zero left-pad).

Sharding: pure batch data-parallel - core i computes batch i. Memory-bound:
all device I/O is bf16 (tolerance 2e-2; bf16 lands ~6e-3). Host pre-casts /
pre-transposes each batch to [C, T+3] with the causal zero-pad baked in, so
channels sit on SBUF partitions and time shifts are free-dim offsets.

Compute is split across engines by a static plan of [128, cols] pieces.
Paths per piece:
  A : PE 4 accumulating diag-matmul taps -> PSUM; ScalarE ACTIVATE evicts
      to bf16 (PE ~1.7ns/col, ACT ~1ns/col).
  B : PE 3 taps -> PSUM; DVE scalar_tensor_tensor does the 4th tap fused
      with the PSUM add + bf16 evict in one op.
  E : ScalarE muls taps 3,2 into temps; DVE STT-folds taps 1,0 + combine.
  D : pure DVE (classic 2 ts-mul + ... or STT chain).
  G : GpSimd muls taps 3,2; DVE STT-folds taps 1,0 + combine.
Weight diag blocks (bf16 [128,128] per (group,tap) used by PE) are loaded
per-group just-in-time; plain weights wt stay f32 (scalar operands are
exempt from DVE dtype speed rules).

All x loads ride the in-order sync HWDGE ring ahead of any store; stores go
on the gpsimd SWDGE ring except the last two, which use the scalar HWDGE
ring (drains independently, ScalarE is idle by then). Dummy matmuls warm
the PE pstate before real tiles arrive.
"""

import numpy as np

B, T, C = 8, 4096, 1024
KTAPS = 4
HALO = KTAPS - 1
CG = 128  # channels per partition-group
N_GROUPS = C // CG
N_CORES = 8
MM_N = 512  # matmul moving free dim = one PSUM bank (f32)
TT_COLS = 2048  # max piece width

# module-level stash so test.py can read profiling info
last_results = None


def _default_plan():
    """[(g, t0, cols, path)] in emission order."""
    return [
        (0, 0, 512, "A"),
        (0, 512, 1536, "A"),
        (1, 0, 2048, "D"),
        (2, 0, 2048, "E"),
        (0, 2048, 2048, "B"),
        (3, 0, 2048, "A"),
        (1, 2048, 2048, "G"),
        (2, 2048, 2048, "B"),
        (4, 0, 2048, "E"),
        (3, 2048, 2048, "A"),
        (5, 0, 2048, "Dc"),
        (4, 2048, 2048, "B"),
        (6, 0, 2048, "A"),
        (5, 2048, 2048, "G"),
        (7, 0, 2048, "B"),
        (6, 2048, 2048, "A"),
        (7, 2048, 1024, "B"),
        (7, 3072, 1024, "A"),
    ]


def _pe_taps(path):
    if path == "A":
        return (3, 2, 1, 0)
    if path == "B":
        return (3, 2, 1)
    return ()


def _wd_layout(plan):
    """Column layout of the diag-block tensor: per PE-using group, the
    union of taps its pieces need, each tap one [CG, CG] block."""
    need = {}  # g -> set of taps
    for g, _, _, path in plan:
        taps = _pe_taps(path)
        if taps:
            need.setdefault(g, set()).update(taps)
    cols = {}  # (g, k) -> col offset
    gcol = {}  # g -> (start, ncols)
    off = 0
    for g in sorted(need):
        start = off
        for k in sorted(need[g]):
            cols[(g, k)] = off
            off += CG
        gcol[g] = (start, off - start)
    return cols, gcol, off


def _build_program(plan=None, xbufs=10, ybufs=8, tbufs=6, psbufs=2):
    import concourse.bass as bass  # noqa: F401
    import concourse.tile as tile
    from concourse import bacc, mybir

    nc = bacc.Bacc(
        "TRN2",
        target_bir_lowering=False,
        debug=False,
        enable_asserts=False,
        num_devices=N_CORES,
    )
    f32 = mybir.dt.float32
    bf16 = mybir.dt.bfloat16
    add = mybir.AluOpType.add
    mult = mybir.AluOpType.mult

    if plan is None:
        plan = _default_plan()
    wd_cols, wd_gcol, wd_ncols = _wd_layout(plan)

    x_ap = nc.dram_tensor("x_t", [C, T + HALO], bf16, kind="ExternalInput").ap()
    w_ap = nc.dram_tensor("w", [CG, N_GROUPS * KTAPS], f32, kind="ExternalInput").ap()
    wd_ap = nc.dram_tensor("wd", [CG, wd_ncols], bf16, kind="ExternalInput").ap()
    out_ap = nc.dram_tensor("out", [C, T], bf16, kind="ExternalOutput").ap()

    # first position each PE group's wd must be resident
    first_pe_pos = {}
    for pos, (g, _, _, path) in enumerate(plan):
        if _pe_taps(path) and g not in first_pe_pos:
            first_pe_pos[g] = pos

    with tile.TileContext(nc) as tc:
        with (
            tc.tile_pool(name="wpool", bufs=1) as wpool,
            tc.tile_pool(name="xpool", bufs=xbufs) as xpool,
            tc.tile_pool(name="ypool", bufs=ybufs) as ypool,
            tc.tile_pool(name="tpool", bufs=tbufs) as tpool,
            tc.tile_pool(name="pspool", bufs=psbufs, space="PSUM") as pspool,
        ):
            # ACT function-table preload via tiny dummy ACTIVATE
            warm = wpool.tile([CG, 1], f32)
            nc.gpsimd.memset(warm[:], 0.0)
            nc.scalar.mul(warm[:], warm[:], 1.0)

            # PE pstate warmup on zeros while first loads are in flight
            wm = wpool.tile([CG, MM_N + CG], bf16)
            nc.gpsimd.memset(wm[:], 0.0)
            ps_w = pspool.tile([CG, TT_COLS], f32, tag="ps")
            for wi in range(4):
                nc.tensor.matmul(
                    ps_w[:, :MM_N],
                    wm[:, :CG],
                    wm[:, CG : CG + MM_N],
                    start=(wi == 0),
                    stop=(wi == 3),
                )
            nc.scalar.mul(warm[:], ps_w[:, :1], 1.0)

            wt = wpool.tile([CG, N_GROUPS * KTAPS], f32)
            wd = wpool.tile([CG, max(wd_ncols, 1)], bf16)
            nc.sync.dma_start(wt[:], w_ap[:])
            wd_loaded = set()

            def load_wd(g):
                if g in wd_loaded or g not in wd_gcol:
                    return
                wd_loaded.add(g)
                s, n = wd_gcol[g]
                nc.sync.dma_start(wd[:, s : s + n], wd_ap[:, s : s + n])

            # all loads hoisted onto the in-order sync ring, plan order,
            # with each group's wd injected just before first needed
            xts = []
            for pos, (g, t0, cols, path) in enumerate(plan):
                for g2, p2 in first_pe_pos.items():
                    if p2 <= pos + 2:
                        load_wd(g2)
                xt = xpool.tile([CG, TT_COLS + HALO], bf16, tag="xt")
                xt = xt[:, : cols + HALO]
                r0 = g * CG
                nc.sync.dma_start(xt[:], x_ap[r0 : r0 + CG, t0 : t0 + cols + HALO])
                xts.append(xt)
            for g in list(first_pe_pos):
                load_wd(g)

            def wcol(g, k):
                return g * KTAPS + k

            for ti, (g, t0, cols, path) in enumerate(plan):
                store_eng = nc.scalar if ti >= len(plan) - 2 else nc.gpsimd
                r0, r1 = g * CG, (g + 1) * CG
                xt = xts[ti]
                yt = ypool.tile([CG, TT_COLS], bf16, tag="yt")
                yt = yt[:, :cols]
                if path in ("A", "B"):
                    taps = _pe_taps(path)
                    ps = pspool.tile([CG, TT_COLS], f32, tag="ps")
                    for ki, k in enumerate(taps):
                        dcol = wd_cols[(g, k)]
                        for c0 in range(0, cols, MM_N):
                            nc.tensor.matmul(
                                ps[:, c0 : c0 + MM_N],
                                wd[:, dcol : dcol + CG],
                                xt[:, c0 + k : c0 + k + MM_N],
                                start=(ki == 0),
                                stop=(ki == len(taps) - 1),
                            )
                    if path == "A":
                        nc.scalar.copy(yt[:], ps[:, :cols])
                    else:  # B: fused last tap + psum add + evict on DVE
                        nc.vector.scalar_tensor_tensor(
                            yt[:],
                            xt[:, 0:cols],
                            wt[:, wcol(g, 0) : wcol(g, 0) + 1],
                            ps[:, :cols],
                            op0=mult,
                            op1=add,
                        )
                elif path == "E" or path == "G":
                    eng = nc.scalar if path == "E" else nc.gpsimd
                    ta = tpool.tile([CG, TT_COLS], bf16, tag="ta")
                    ta = ta[:, :cols]
                    tb = tpool.tile([CG, TT_COLS], bf16, tag="tb")
                    tb = tb[:, :cols]
                    tcv = tpool.tile([CG, TT_COLS], bf16, tag="tc")
                    tcv = tcv[:, :cols]
                    if path == "E":
                        eng.mul(
                            ta[:], xt[:, HALO : HALO + cols],
                            wt[:, wcol(g, 3) : wcol(g, 3) + 1],
                        )
                        eng.mul(
                            tb[:], xt[:, 2 : 2 + cols],
                            wt[:, wcol(g, 2) : wcol(g, 2) + 1],
                        )
                    else:
                        eng.tensor_scalar_mul(
                            ta[:], xt[:, HALO : HALO + cols],
                            wt[:, wcol(g, 3) : wcol(g, 3) + 1],
                        )
                        eng.tensor_scalar_mul(
                            tb[:], xt[:, 2 : 2 + cols],
                            wt[:, wcol(g, 2) : wcol(g, 2) + 1],
                        )
                    # DVE: fold taps 1,0 with adds via STT, final combine
                    nc.vector.scalar_tensor_tensor(
                        tcv[:], xt[:, 1 : 1 + cols],
                        wt[:, wcol(g, 1) : wcol(g, 1) + 1], ta[:],
                        op0=mult, op1=add,
                    )
                    nc.vector.scalar_tensor_tensor(
                        yt[:], xt[:, 0:cols],
                        wt[:, wcol(g, 0) : wcol(g, 0) + 1], tb[:],
                        op0=mult, op1=add,
                    )
                    nc.vector.tensor_tensor(yt[:], yt[:], tcv[:], op=add)
                elif path == "D":  # classic pure-DVE
                    ta = tpool.tile([CG, TT_COLS], bf16, tag="ta")
                    ta = ta[:, :cols]
                    tb = tpool.tile([CG, TT_COLS], bf16, tag="tb")
                    tb = tb[:, :cols]
                    tcv = tpool.tile([CG, TT_COLS], bf16, tag="tc")
                    tcv = tcv[:, :cols]
                    nc.vector.tensor_scalar_mul(
                        ta[:], xt[:, HALO : HALO + cols],
                        wt[:, wcol(g, 3) : wcol(g, 3) + 1],
                    )
                    nc.vector.tensor_scalar_mul(
                        tb[:], xt[:, 2 : 2 + cols],
                        wt[:, wcol(g, 2) : wcol(g, 2) + 1],
                    )
                    nc.vector.tensor_scalar_mul(
                        tcv[:], xt[:, 1 : 1 + cols],
                        wt[:, wcol(g, 1) : wcol(g, 1) + 1],
                    )
                    nc.vector.tensor_scalar_mul(
                        yt[:], xt[:, 0:cols], wt[:, wcol(g, 0) : wcol(g, 0) + 1]
                    )
                    nc.vector.tensor_tensor(ta[:], ta[:], tb[:], op=add)
                    nc.vector.tensor_tensor(yt[:], yt[:], tcv[:], op=add)
                    nc.vector.tensor_tensor(yt[:], yt[:], ta[:], op=add)
                elif path == "Dc":  # STT chain pure-DVE
                    ta = tpool.tile([CG, TT_COLS], bf16, tag="ta")
                    ta = ta[:, :cols]
                    tb = tpool.tile([CG, TT_COLS], bf16, tag="tb")
                    tb = tb[:, :cols]
                    tcv = tpool.tile([CG, TT_COLS], bf16, tag="tc")
                    tcv = tcv[:, :cols]
                    nc.vector.tensor_scalar_mul(
                        ta[:], xt[:, HALO : HALO + cols],
                        wt[:, wcol(g, 3) : wcol(g, 3) + 1],
                    )
                    for k, src, dst in ((2, ta, tb), (1, tb, tcv), (0, tcv, yt)):
                        nc.vector.scalar_tensor_tensor(
                            dst[:], xt[:, k : k + cols],
                            wt[:, wcol(g, k) : wcol(g, k) + 1], src[:],
                            op0=mult, op1=add,
                        )
                else:
                    raise ValueError(path)
                store_eng.dma_start(out_ap[r0:r1, t0 : t0 + cols], yt[:])
    nc.compile()
    return nc


def _prep_weights(W: np.ndarray) -> np.ndarray:
    # wt[p, g*KTAPS + k] = W[k, 0, g*CG + p]
    wk = W.reshape(KTAPS, N_GROUPS, CG)  # [k, g, p]
    return np.ascontiguousarray(
        wk.transpose(2, 1, 0).reshape(CG, N_GROUPS * KTAPS).astype(np.float32)
    )


def _prep_diag(W: np.ndarray, plan, bf16) -> np.ndarray:
    wd_cols, _, wd_ncols = _wd_layout(plan)
    wd = np.zeros((CG, max(wd_ncols, 1)), dtype=bf16)
    for (g, k), off in wd_cols.items():
        np.fill_diagonal(
            wd[:, off : off + CG], W[k, 0, g * CG : (g + 1) * CG].astype(bf16)
        )
    return wd


def kernel(x: np.ndarray, W: np.ndarray) -> np.ndarray:
    global last_results
    import ml_dtypes
    from concourse.bass_utils import run_bass_kernel_spmd

    bf16 = ml_dtypes.bfloat16
    x = np.asarray(x, dtype=np.float32)
    W = np.asarray(W, dtype=np.float32)
    assert x.shape == (B, T, C) and W.shape == (KTAPS, 1, C)

    plan = _default_plan()
    nc = _build_program(plan=plan)
    wt = _prep_weights(W)
    wd = _prep_diag(W, plan, bf16)
    x_bf = x.astype(bf16)
    zpad = np.zeros((C, HALO), dtype=bf16)
    in_maps = [
        {
            # [C, T+HALO] bf16, causal zero left-pad baked in
            "x_t": np.ascontiguousarray(
                np.concatenate([zpad, x_bf[i].T], axis=1)
            ),
            "w": wt,
            "wd": wd,
        }
        for i in range(N_CORES)
    ]
    import os

    trace = False
    if os.environ.get("BASS_TRACE") and not os.environ.get("BASS_NEVER_TRACE"):
        try:
            import antenv.axon_hooks  # noqa: F401

            trace = True
        except ImportError:
            os.environ["BASS_NEVER_TRACE"] = "1"
    res = run_bass_kernel_spmd(
        nc, in_maps, core_ids=list(range(N_CORES)), trace=trace
    )
    last_results = res
    y = np.stack(
        [np.asarray(res.results[i]["out"]).astype(np.float32).T for i in range(N_CORES)]
    )
    return np.ascontiguousarray(y)


# revision 6
# speedup vs baseline: 2.6289x; 2.6289x over previous
"""Depthwise causal conv1d (K=4, dilation=1) on 8 TRN2 NeuronCores.

Reference: x [B=8, T=4096, C=1024] f32, W [4, 1, 1024] f32 (WIO layout),
y[b, t, c] = sum_k W[k, 0, c] * x[b, t - 3 + k, c]  (zero left-pad).

Sharding: pure batch data-parallel - core i computes batch i. Memory-bound:
all device I/O is bf16 (tolerance 2e-2; bf16 lands ~6e-3). Host pre-casts /
pre-transposes each batch to [C, T+3] with the causal zero-pad baked in, so
channels sit on SBUF partitions and time shifts are free-dim offsets.

Compute is split across engines by a static plan of [128, cols] pieces.
Paths per piece:
  A : PE 4 accumulating diag-matmul taps -> PSUM; ScalarE ACTIVATE evicts
      to bf16 (PE ~1.7ns/col, ACT ~1ns/col).
  B : PE 3 taps -> PSUM; DVE scalar_tensor_tensor does the 4th tap fused
      with the PSUM add + bf16 evict in one op.
  E : ScalarE muls taps 3,2 into temps; DVE STT-folds taps 1,0 + combine.
  D : pure DVE (classic 2 ts-mul + ... or STT chain).
  G : GpSimd muls taps 3,2; DVE STT-folds taps 1,0 + combine.
Weight diag blocks (bf16 [128,128] per (group,tap) used by PE) are loaded
per-group just-in-time; plain weights wt stay f32 (scalar operands are
exempt from DVE dtype speed rules).

All x loads ride the in-order sync HWDGE ring ahead of any store; stores go
on the gpsimd SWDGE ring except the last two, which use the scalar HWDGE
ring (drains independently, ScalarE is idle by then). Dummy matmuls warm
the PE pstate before real tiles arrive.
"""

import numpy as np

B, T, C = 8, 4096, 1024
KTAPS = 4
HALO = KTAPS - 1
CG = 128  # channels per partition-group
N_GROUPS = C // CG
N_CORES = 8
MM_N = 512  # matmul moving free dim = one PSUM bank (f32)
PS_N = 1024  # PSUM sub-piece width (2 banks)
TT_COLS = 2048  # max piece width

# module-level stash so test.py can read profiling info
last_results = None


def _default_plan():
    """[(g, t0, cols, path)] in emission order.

    Engine budgets (measured rates, per 2048-col piece):
      A: PE 3.5us + Scalar 2.08us   B: PE 2.63us + DVE 2.34us (STT evict)
      E: Scalar 4.16us + DVE 5.15us D: DVE 6.63us
    Totals here: PE ~36.8, Scalar ~28, DVE ~34.8 -> all inside the ~37us
    compute window of a ~50us DMA-bound schedule. No GpSimd compute: its
    tensor ops run ~25x slow AND stall concurrent DVE ops.
    """
    return [
        (0, 0, 512, "A"),
        (1, 0, 1024, "D"),
        (0, 512, 2048, "A"),
        (2, 0, 2048, "E"),
        (1, 1024, 1024, "D"),
        (3, 0, 2048, "A"),
        (2, 2048, 2048, "B"),
        (4, 0, 2048, "E"),
        (0, 2560, 1536, "A"),
        (3, 2048, 2048, "B"),
        (5, 0, 2048, "A"),
        (4, 2048, 2048, "E"),
        (1, 2048, 2048, "B"),
        (6, 0, 2048, "A"),
        (5, 2048, 2048, "B"),
        (7, 0, 2048, "A"),
        (6, 2048, 2048, "A"),
        (7, 2048, 1024, "D"),
        (7, 3072, 1024, "A"),
    ]


def _pe_taps(path):
    if path == "A":
        return (3, 2, 1, 0)
    if path == "B":
        return (3, 2, 1)
    return ()


def _wd_layout(plan):
    """Column layout of the diag-block tensor: per PE-using group, the
    union of taps its pieces need, each tap one [CG, CG] block."""
    need = {}  # g -> set of taps
    for g, _, _, path in plan:
        taps = _pe_taps(path)
        if taps:
            need.setdefault(g, set()).update(taps)
    cols = {}  # (g, k) -> col offset
    gcol = {}  # g -> (start, ncols)
    off = 0
    for g in sorted(need):
        start = off
        for k in sorted(need[g]):
            cols[(g, k)] = off
            off += CG
        gcol[g] = (start, off - start)
    return cols, gcol, off


def _build_program(plan=None, xbufs=10, ybufs=8, tbufs=6, psbufs=4):
    import concourse.bass as bass  # noqa: F401
    import concourse.tile as tile
    from concourse import bacc, mybir

    nc = bacc.Bacc(
        "TRN2",
        target_bir_lowering=False,
        debug=False,
        enable_asserts=False,
        num_devices=N_CORES,
    )
    f32 = mybir.dt.float32
    bf16 = mybir.dt.bfloat16
    add = mybir.AluOpType.add
    mult = mybir.AluOpType.mult

    if plan is None:
        plan = _default_plan()
    wd_cols, wd_gcol, wd_ncols = _wd_layout(plan)

    x_ap = nc.dram_tensor("x_t", [C, T + HALO], bf16, kind="ExternalInput").ap()
    w_ap = nc.dram_tensor("w", [CG, N_GROUPS * KTAPS], f32, kind="ExternalInput").ap()
    wd_ap = nc.dram_tensor("wd", [CG, wd_ncols], bf16, kind="ExternalInput").ap()
    out_ap = nc.dram_tensor("out", [C, T], bf16, kind="ExternalOutput").ap()

    # first position each PE group's wd must be resident
    first_pe_pos = {}
    for pos, (g, _, _, path) in enumerate(plan):
        if _pe_taps(path) and g not in first_pe_pos:
            first_pe_pos[g] = pos

    with tile.TileContext(nc) as tc:
        with (
            tc.tile_pool(name="wpool", bufs=1) as wpool,
            tc.tile_pool(name="xpool", bufs=xbufs) as xpool,
            tc.tile_pool(name="ypool", bufs=ybufs) as ypool,
            tc.tile_pool(name="tpool", bufs=tbufs) as tpool,
            tc.tile_pool(name="pspool", bufs=psbufs, space="PSUM") as pspool,
        ):
            # ACT function-table preload via tiny dummy ACTIVATE
            warm = wpool.tile([CG, 1], f32)
            nc.gpsimd.memset(warm[:], 0.0)
            nc.scalar.mul(warm[:], warm[:], 1.0)

            # PE pstate warmup on zeros while first loads are in flight
            wm = wpool.tile([CG, MM_N + CG], bf16)
            nc.gpsimd.memset(wm[:], 0.0)
            ps_w = pspool.tile([CG, PS_N], f32, tag="ps")
            for wi in range(4):
                nc.tensor.matmul(
                    ps_w[:, :MM_N],
                    wm[:, :CG],
                    wm[:, CG : CG + MM_N],
                    start=(wi == 0),
                    stop=(wi == 3),
                )
            nc.scalar.mul(warm[:], ps_w[:, :1], 1.0)

            wt = wpool.tile([CG, N_GROUPS * KTAPS], f32)
            wd = wpool.tile([CG, max(wd_ncols, 1)], bf16)
            nc.sync.dma_start(wt[:], w_ap[:])
            wd_loaded = set()

            def load_wd(g):
                if g in wd_loaded or g not in wd_gcol:
                    return
                wd_loaded.add(g)
                s, n = wd_gcol[g]
                nc.sync.dma_start(wd[:, s : s + n], wd_ap[:, s : s + n])

            # all loads hoisted onto the in-order sync ring, plan order,
            # with each group's wd injected just before first needed
            xts = []
            for pos, (g, t0, cols, path) in enumerate(plan):
                for g2, p2 in first_pe_pos.items():
                    if p2 <= pos + 2:
                        load_wd(g2)
                xt = xpool.tile([CG, TT_COLS + HALO], bf16, tag="xt")
                xt = xt[:, : cols + HALO]
                r0 = g * CG
                nc.sync.dma_start(xt[:], x_ap[r0 : r0 + CG, t0 : t0 + cols + HALO])
                xts.append(xt)
            for g in list(first_pe_pos):
                load_wd(g)

            def wcol(g, k):
                return g * KTAPS + k

            for ti, (g, t0, cols, path) in enumerate(plan):
                store_eng = nc.scalar if ti >= len(plan) - 2 else nc.gpsimd
                r0, r1 = g * CG, (g + 1) * CG
                xt = xts[ti]
                yt = ypool.tile([CG, TT_COLS], bf16, tag="yt")
                yt = yt[:, :cols]
                if path in ("A", "B"):
                    taps = _pe_taps(path)
                    # 1024-col PSUM sub-pieces (2 banks each, 4 pool bufs)
                    # keep the PE->evict relay deep so neither engine stalls
                    for p0 in range(0, cols, PS_N):
                        pn = min(PS_N, cols - p0)
                        ps = pspool.tile([CG, PS_N], f32, tag="ps")
                        for ki, k in enumerate(taps):
                            dcol = wd_cols[(g, k)]
                            for c0 in range(p0, p0 + pn, MM_N):
                                nc.tensor.matmul(
                                    ps[:, c0 - p0 : c0 - p0 + MM_N],
                                    wd[:, dcol : dcol + CG],
                                    xt[:, c0 + k : c0 + k + MM_N],
                                    start=(ki == 0),
                                    stop=(ki == len(taps) - 1),
                                )
                        if path == "A":
                            nc.scalar.copy(yt[:, p0 : p0 + pn], ps[:, :pn])
                        else:  # B: fused last tap + psum add + evict on DVE
                            nc.vector.scalar_tensor_tensor(
                                yt[:, p0 : p0 + pn],
                                xt[:, p0 : p0 + pn],
                                wt[:, wcol(g, 0) : wcol(g, 0) + 1],
                                ps[:, :pn],
                                op0=mult,
                                op1=add,
                            )
                elif path == "E":
                    ta = tpool.tile([CG, TT_COLS], bf16, tag="ta")
                    ta = ta[:, :cols]
                    tb = tpool.tile([CG, TT_COLS], bf16, tag="tb")
                    tb = tb[:, :cols]
                    tcv = tpool.tile([CG, TT_COLS], bf16, tag="tc")
                    tcv = tcv[:, :cols]
                    nc.scalar.mul(
                        ta[:], xt[:, HALO : HALO + cols],
                        wt[:, wcol(g, 3) : wcol(g, 3) + 1],
                    )
                    nc.scalar.mul(
                        tb[:], xt[:, 2 : 2 + cols],
                        wt[:, wcol(g, 2) : wcol(g, 2) + 1],
                    )
                    # DVE: taps 1,0 (ts 4x) + 3 tensor_tensor adds (2x)
                    nc.vector.tensor_scalar_mul(
                        tcv[:], xt[:, 1 : 1 + cols],
                        wt[:, wcol(g, 1) : wcol(g, 1) + 1],
                    )
                    nc.vector.tensor_scalar_mul(
                        yt[:], xt[:, 0:cols], wt[:, wcol(g, 0) : wcol(g, 0) + 1]
                    )
                    nc.vector.tensor_tensor(ta[:], ta[:], tb[:], op=add)
                    nc.vector.tensor_tensor(yt[:], yt[:], tcv[:], op=add)
                    nc.vector.tensor_tensor(yt[:], yt[:], ta[:], op=add)
                elif path == "D":  # classic pure-DVE
                    ta = tpool.tile([CG, TT_COLS], bf16, tag="ta")
                    ta = ta[:, :cols]
                    tb = tpool.tile([CG, TT_COLS], bf16, tag="tb")
                    tb = tb[:, :cols]
                    tcv = tpool.tile([CG, TT_COLS], bf16, tag="tc")
                    tcv = tcv[:, :cols]
                    nc.vector.tensor_scalar_mul(
                        ta[:], xt[:, HALO : HALO + cols],
                        wt[:, wcol(g, 3) : wcol(g, 3) + 1],
                    )
                    nc.vector.tensor_scalar_mul(
                        tb[:], xt[:, 2 : 2 + cols],
                        wt[:, wcol(g, 2) : wcol(g, 2) + 1],
                    )
                    nc.vector.tensor_scalar_mul(
                        tcv[:], xt[:, 1 : 1 + cols],
                        wt[:, wcol(g, 1) : wcol(g, 1) + 1],
                    )
                    nc.vector.tensor_scalar_mul(
                        yt[:], xt[:, 0:cols], wt[:, wcol(g, 0) : wcol(g, 0) + 1]
                    )
                    nc.vector.tensor_tensor(ta[:], ta[:], tb[:], op=add)
                    nc.vector.tensor_tensor(yt[:], yt[:], tcv[:], op=add)
                    nc.vector.tensor_tensor(yt[:], yt[:], ta[:], op=add)
                elif path == "Dc":  # STT chain pure-DVE
                    ta = tpool.tile([CG, TT_COLS], bf16, tag="ta")
                    ta = ta[:, :cols]
                    tb = tpool.tile([CG, TT_COLS], bf16, tag="tb")
                    tb = tb[:, :cols]
                    tcv = tpool.tile([CG, TT_COLS], bf16, tag="tc")
                    tcv = tcv[:, :cols]
                    nc.vector.tensor_scalar_mul(
                        ta[:], xt[:, HALO : HALO + cols],
                        wt[:, wcol(g, 3) : wcol(g, 3) + 1],
                    )
                    for k, src, dst in ((2, ta, tb), (1, tb, tcv), (0, tcv, yt)):
                        nc.vector.scalar_tensor_tensor(
                            dst[:], xt[:, k : k + cols],
                            wt[:, wcol(g, k) : wcol(g, k) + 1], src[:],
                            op0=mult, op1=add,
                        )
                else:
                    raise ValueError(path)
                store_eng.dma_start(out_ap[r0:r1, t0 : t0 + cols], yt[:])
    nc.compile()
    return nc


def _prep_weights(W: np.ndarray) -> np.ndarray:
    # wt[p, g*KTAPS + k] = W[k, 0, g*CG + p]
    wk = W.reshape(KTAPS, N_GROUPS, CG)  # [k, g, p]
    return np.ascontiguousarray(
        wk.transpose(2, 1, 0).reshape(CG, N_GROUPS * KTAPS).astype(np.float32)
    )


def _prep_diag(W: np.ndarray, plan, bf16) -> np.ndarray:
    wd_cols, _, wd_ncols = _wd_layout(plan)
    wd = np.zeros((CG, max(wd_ncols, 1)), dtype=bf16)
    for (g, k), off in wd_cols.items():
        np.fill_diagonal(
            wd[:, off : off + CG], W[k, 0, g * CG : (g + 1) * CG].astype(bf16)
        )
    return wd


def kernel(x: np.ndarray, W: np.ndarray) -> np.ndarray:
    global last_results
    import ml_dtypes
    from concourse.bass_utils import run_bass_kernel_spmd

    bf16 = ml_dtypes.bfloat16
    x = np.asarray(x, dtype=np.float32)
    W = np.asarray(W, dtype=np.float32)
    assert x.shape == (B, T, C) and W.shape == (KTAPS, 1, C)

    plan = _default_plan()
    nc = _build_program(plan=plan)
    wt = _prep_weights(W)
    wd = _prep_diag(W, plan, bf16)
    x_bf = x.astype(bf16)
    zpad = np.zeros((C, HALO), dtype=bf16)
    in_maps = [
        {
            # [C, T+HALO] bf16, causal zero left-pad baked in
            "x_t": np.ascontiguousarray(
                np.concatenate([zpad, x_bf[i].T], axis=1)
            ),
            "w": wt,
            "wd": wd,
        }
        for i in range(N_CORES)
    ]
    import os

    trace = False
    if os.environ.get("BASS_TRACE") and not os.environ.get("BASS_NEVER_TRACE"):
        try:
            import antenv.axon_hooks  # noqa: F401

            trace = True
        except ImportError:
            os.environ["BASS_NEVER_TRACE"] = "1"
    res = run_bass_kernel_spmd(
        nc, in_maps, core_ids=list(range(N_CORES)), trace=trace
    )
    last_results = res
    y = np.stack(
        [np.asarray(res.results[i]["out"]).astype(np.float32).T for i in range(N_CORES)]
    )
    return np.ascontiguousarray(y)


# revision 7
# speedup vs baseline: 2.7855x; 1.0596x over previous
"""Depthwise causal conv1d (K=4, dilation=1) on 8 TRN2 NeuronCores.

Reference: x [B=8, T=4096, C=1024] f32, W [4, 1, 1024] f32 (WIO layout),
y[b, t, c] = sum_k W[k, 0, c] * x[b, t - 3 + k, c]  (zero left-pad).

Sharding: pure batch data-parallel - core i computes batch i. Memory-bound:
all device I/O is bf16 (tolerance 2e-2; bf16 lands ~6e-3). Host pre-casts /
pre-transposes each batch to [C, T+3] with the causal zero-pad baked in, so
channels sit on SBUF partitions and time shifts are free-dim offsets.

Compute is split across engines by a static plan of [128, cols] pieces.
Paths per piece:
  A : PE 4 accumulating diag-matmul taps -> PSUM; ScalarE ACTIVATE evicts
      to bf16 (PE ~1.7ns/col, ACT ~1ns/col).
  B : PE 3 taps -> PSUM; DVE scalar_tensor_tensor does the 4th tap fused
      with the PSUM add + bf16 evict in one op.
  E : ScalarE muls taps 3,2 into temps; DVE STT-folds taps 1,0 + combine.
  D : pure DVE (classic 2 ts-mul + ... or STT chain).
  G : GpSimd muls taps 3,2; DVE STT-folds taps 1,0 + combine.
Weight diag blocks (bf16 [128,128] per (group,tap) used by PE) are loaded
per-group just-in-time; plain weights wt stay f32 (scalar operands are
exempt from DVE dtype speed rules).

All x loads ride the in-order sync HWDGE ring ahead of any store; stores go
on the gpsimd SWDGE ring except the last two, which use the scalar HWDGE
ring (drains independently, ScalarE is idle by then). Dummy matmuls warm
the PE pstate before real tiles arrive.
"""

import numpy as np

B, T, C = 8, 4096, 1024
KTAPS = 4
HALO = KTAPS - 1
CG = 128  # channels per partition-group
N_GROUPS = C // CG
N_CORES = 8
MM_N = 512  # matmul moving free dim = one PSUM bank (f32)
PS_N = 1024  # PSUM sub-piece width (2 banks)
TT_COLS = 2048  # max piece width

# module-level stash so test.py can read profiling info
last_results = None


def _default_plan():
    """[(g, t0, cols, path)] in emission order.

    Engine budgets (measured rates, per 2048-col piece):
      A: PE 3.5us + Scalar 2.08us   B: PE 2.63us + DVE 2.34us (STT evict)
      E: Scalar 4.16us + DVE 5.15us D: DVE 6.63us
    Totals here: PE ~36.8, Scalar ~28, DVE ~34.8 -> all inside the ~37us
    compute window of a ~50us DMA-bound schedule. No GpSimd compute: its
    tensor ops run ~25x slow AND stall concurrent DVE ops.
    """
    return [
        (1, 0, 1024, "D"),
        (0, 0, 512, "A"),
        (0, 512, 2048, "A"),
        (2, 0, 2048, "E"),
        (1, 1024, 1024, "D"),
        (3, 0, 2048, "A"),
        (2, 2048, 2048, "B"),
        (4, 0, 2048, "E"),
        (0, 2560, 1536, "A"),
        (3, 2048, 2048, "B"),
        (5, 0, 2048, "A"),
        (4, 2048, 2048, "E"),
        (1, 2048, 2048, "B"),
        (6, 0, 2048, "A"),
        (5, 2048, 2048, "B"),
        (7, 0, 2048, "A"),
        (6, 2048, 2048, "A"),
        (7, 2048, 1536, "B"),
        (7, 3584, 512, "A"),
    ]


def _pe_taps(path):
    if path == "A":
        return (3, 2, 1, 0)
    if path == "B":
        return (3, 2, 1)
    return ()


def _wd_layout(plan):
    """Column layout of the diag-block tensor: per PE-using group, the
    union of taps its pieces need, each tap one [CG, CG] block."""
    need = {}  # g -> set of taps
    for g, _, _, path in plan:
        taps = _pe_taps(path)
        if taps:
            need.setdefault(g, set()).update(taps)
    cols = {}  # (g, k) -> col offset
    gcol = {}  # g -> (start, ncols)
    off = 0
    for g in sorted(need):
        start = off
        for k in sorted(need[g]):
            cols[(g, k)] = off
            off += CG
        gcol[g] = (start, off - start)
    return cols, gcol, off


def _build_program(plan=None, xbufs=10, ybufs=8, tbufs=6, psbufs=2):
    import concourse.bass as bass  # noqa: F401
    import concourse.tile as tile
    from concourse import bacc, mybir

    nc = bacc.Bacc(
        "TRN2",
        target_bir_lowering=False,
        debug=False,
        enable_asserts=False,
        num_devices=N_CORES,
    )
    f32 = mybir.dt.float32
    bf16 = mybir.dt.bfloat16
    add = mybir.AluOpType.add
    mult = mybir.AluOpType.mult

    if plan is None:
        plan = _default_plan()
    wd_cols, wd_gcol, wd_ncols = _wd_layout(plan)

    x_ap = nc.dram_tensor("x_t", [C, T + HALO], bf16, kind="ExternalInput").ap()
    w_ap = nc.dram_tensor("w", [CG, N_GROUPS * KTAPS], f32, kind="ExternalInput").ap()
    wd_ap = nc.dram_tensor("wd", [CG, wd_ncols], bf16, kind="ExternalInput").ap()
    out_ap = nc.dram_tensor("out", [C, T], bf16, kind="ExternalOutput").ap()

    # first position each PE group's wd must be resident
    first_pe_pos = {}
    for pos, (g, _, _, path) in enumerate(plan):
        if _pe_taps(path) and g not in first_pe_pos:
            first_pe_pos[g] = pos

    with tile.TileContext(nc) as tc:
        with (
            tc.tile_pool(name="wpool", bufs=1) as wpool,
            tc.tile_pool(name="xpool", bufs=xbufs) as xpool,
            tc.tile_pool(name="ypool", bufs=ybufs) as ypool,
            tc.tile_pool(name="tpool", bufs=tbufs) as tpool,
            tc.tile_pool(name="pspool", bufs=psbufs, space="PSUM") as pspool,
        ):
            # ACT function-table preload via tiny dummy ACTIVATE
            warm = wpool.tile([CG, 1], f32)
            nc.gpsimd.memset(warm[:], 0.0)
            nc.scalar.mul(warm[:], warm[:], 1.0)

            # PE pstate warmup on zeros while first loads are in flight
            wm = wpool.tile([CG, MM_N + CG], bf16)
            nc.gpsimd.memset(wm[:], 0.0)
            ps_w = pspool.tile([CG, TT_COLS], f32, tag="ps")
            for wi in range(4):
                nc.tensor.matmul(
                    ps_w[:, :MM_N],
                    wm[:, :CG],
                    wm[:, CG : CG + MM_N],
                    start=(wi == 0),
                    stop=(wi == 3),
                )
            nc.scalar.mul(warm[:], ps_w[:, :1], 1.0)

            wt = wpool.tile([CG, N_GROUPS * KTAPS], f32)
            wd = wpool.tile([CG, max(wd_ncols, 1)], bf16)
            nc.sync.dma_start(wt[:], w_ap[:])
            wd_loaded = set()

            def load_wd(g):
                if g in wd_loaded or g not in wd_gcol:
                    return
                wd_loaded.add(g)
                s, n = wd_gcol[g]
                nc.sync.dma_start(wd[:, s : s + n], wd_ap[:, s : s + n])

            # all loads hoisted onto the in-order sync ring, plan order,
            # with each group's wd injected just before first needed
            xts = []
            for pos, (g, t0, cols, path) in enumerate(plan):
                for g2, p2 in first_pe_pos.items():
                    if p2 <= pos + 2:
                        load_wd(g2)
                xt = xpool.tile([CG, TT_COLS + HALO], bf16, tag="xt")
                xt = xt[:, : cols + HALO]
                r0 = g * CG
                nc.sync.dma_start(xt[:], x_ap[r0 : r0 + CG, t0 : t0 + cols + HALO])
                xts.append(xt)
            for g in list(first_pe_pos):
                load_wd(g)

            def wcol(g, k):
                return g * KTAPS + k

            for ti, (g, t0, cols, path) in enumerate(plan):
                store_eng = (
                    nc.sync if ti == len(plan) - 1
                    else nc.scalar if ti == len(plan) - 2
                    else nc.gpsimd
                )
                r0, r1 = g * CG, (g + 1) * CG
                xt = xts[ti]
                yt = ypool.tile([CG, TT_COLS], bf16, tag="yt")
                yt = yt[:, :cols]
                if path in ("A", "B"):
                    taps = _pe_taps(path)
                    # k-outer over the whole piece: stationary swaps once per
                    # tap so matmuls pipeline at full rate (~0.43ns/col/tap)
                    ps = pspool.tile([CG, TT_COLS], f32, tag="ps")
                    for ki, k in enumerate(taps):
                        dcol = wd_cols[(g, k)]
                        for c0 in range(0, cols, MM_N):
                            nc.tensor.matmul(
                                ps[:, c0 : c0 + MM_N],
                                wd[:, dcol : dcol + CG],
                                xt[:, c0 + k : c0 + k + MM_N],
                                start=(ki == 0),
                                stop=(ki == len(taps) - 1),
                            )
                    if path == "A":
                        nc.scalar.copy(yt[:], ps[:, :cols])
                    else:  # B: fused last tap + psum add + evict on DVE
                        nc.vector.scalar_tensor_tensor(
                            yt[:],
                            xt[:, 0:cols],
                            wt[:, wcol(g, 0) : wcol(g, 0) + 1],
                            ps[:, :cols],
                            op0=mult,
                            op1=add,
                        )
                elif path == "E":
                    ta = tpool.tile([CG, TT_COLS], bf16, tag="ta")
                    ta = ta[:, :cols]
                    tb = tpool.tile([CG, TT_COLS], bf16, tag="tb")
                    tb = tb[:, :cols]
                    tcv = tpool.tile([CG, TT_COLS], bf16, tag="tc")
                    tcv = tcv[:, :cols]
                    nc.scalar.mul(
                        ta[:], xt[:, HALO : HALO + cols],
                        wt[:, wcol(g, 3) : wcol(g, 3) + 1],
                    )
                    nc.scalar.mul(
                        tb[:], xt[:, 2 : 2 + cols],
                        wt[:, wcol(g, 2) : wcol(g, 2) + 1],
                    )
                    # DVE: taps 1,0 (ts 4x) + 3 tensor_tensor adds (2x)
                    nc.vector.tensor_scalar_mul(
                        tcv[:], xt[:, 1 : 1 + cols],
                        wt[:, wcol(g, 1) : wcol(g, 1) + 1],
                    )
                    nc.vector.tensor_scalar_mul(
                        yt[:], xt[:, 0:cols], wt[:, wcol(g, 0) : wcol(g, 0) + 1]
                    )
                    nc.vector.tensor_tensor(ta[:], ta[:], tb[:], op=add)
                    nc.vector.tensor_tensor(yt[:], yt[:], tcv[:], op=add)
                    nc.vector.tensor_tensor(yt[:], yt[:], ta[:], op=add)
                elif path == "D":  # classic pure-DVE
                    ta = tpool.tile([CG, TT_COLS], bf16, tag="ta")
                    ta = ta[:, :cols]
                    tb = tpool.tile([CG, TT_COLS], bf16, tag="tb")
                    tb = tb[:, :cols]
                    tcv = tpool.tile([CG, TT_COLS], bf16, tag="tc")
                    tcv = tcv[:, :cols]
                    nc.vector.tensor_scalar_mul(
                        ta[:], xt[:, HALO : HALO + cols],
                        wt[:, wcol(g, 3) : wcol(g, 3) + 1],
                    )
                    nc.vector.tensor_scalar_mul(
                        tb[:], xt[:, 2 : 2 + cols],
                        wt[:, wcol(g, 2) : wcol(g, 2) + 1],
                    )
                    nc.vector.tensor_scalar_mul(
                        tcv[:], xt[:, 1 : 1 + cols],
                        wt[:, wcol(g, 1) : wcol(g, 1) + 1],
                    )
                    nc.vector.tensor_scalar_mul(
                        yt[:], xt[:, 0:cols], wt[:, wcol(g, 0) : wcol(g, 0) + 1]
                    )
                    nc.vector.tensor_tensor(ta[:], ta[:], tb[:], op=add)
                    nc.vector.tensor_tensor(yt[:], yt[:], tcv[:], op=add)
                    nc.vector.tensor_tensor(yt[:], yt[:], ta[:], op=add)
                elif path == "Dc":  # STT chain pure-DVE
                    ta = tpool.tile([CG, TT_COLS], bf16, tag="ta")
                    ta = ta[:, :cols]
                    tb = tpool.tile([CG, TT_COLS], bf16, tag="tb")
                    tb = tb[:, :cols]
                    tcv = tpool.tile([CG, TT_COLS], bf16, tag="tc")
                    tcv = tcv[:, :cols]
                    nc.vector.tensor_scalar_mul(
                        ta[:], xt[:, HALO : HALO + cols],
                        wt[:, wcol(g, 3) : wcol(g, 3) + 1],
                    )
                    for k, src, dst in ((2, ta, tb), (1, tb, tcv), (0, tcv, yt)):
                        nc.vector.scalar_tensor_tensor(
                            dst[:], xt[:, k : k + cols],
                            wt[:, wcol(g, k) : wcol(g, k) + 1], src[:],
                            op0=mult, op1=add,
                        )
                else:
                    raise ValueError(path)
                store_eng.dma_start(out_ap[r0:r1, t0 : t0 + cols], yt[:])
    nc.compile()
    return nc


def _prep_weights(W: np.ndarray) -> np.ndarray:
    # wt[p, g*KTAPS + k] = W[k, 0, g*CG + p]
    wk = W.reshape(KTAPS, N_GROUPS, CG)  # [k, g, p]
    return np.ascontiguousarray(
        wk.transpose(2, 1, 0).reshape(CG, N_GROUPS * KTAPS).astype(np.float32)
    )


def _prep_diag(W: np.ndarray, plan, bf16) -> np.ndarray:
    wd_cols, _, wd_ncols = _wd_layout(plan)
    wd = np.zeros((CG, max(wd_ncols, 1)), dtype=bf16)
    for (g, k), off in wd_cols.items():
        np.fill_diagonal(
            wd[:, off : off + CG], W[k, 0, g * CG : (g + 1) * CG].astype(bf16)
        )
    return wd


def kernel(x: np.ndarray, W: np.ndarray) -> np.ndarray:
    global last_results
    import ml_dtypes
    from concourse.bass_utils import run_bass_kernel_spmd

    bf16 = ml_dtypes.bfloat16
    x = np.asarray(x, dtype=np.float32)
    W = np.asarray(W, dtype=np.float32)
    assert x.shape == (B, T, C) and W.shape == (KTAPS, 1, C)

    plan = _default_plan()
    nc = _build_program(plan=plan)
    wt = _prep_weights(W)
    wd = _prep_diag(W, plan, bf16)
    x_bf = x.astype(bf16)
    zpad = np.zeros((C, HALO), dtype=bf16)
    in_maps = [
        {
            # [C, T+HALO] bf16, causal zero left-pad baked in
            "x_t": np.ascontiguousarray(
                np.concatenate([zpad, x_bf[i].T], axis=1)
            ),
            "w": wt,
            "wd": wd,
        }
        for i in range(N_CORES)
    ]
    import os

    trace = False
    if os.environ.get("BASS_TRACE") and not os.environ.get("BASS_NEVER_TRACE"):
        try:
            import antenv.axon_hooks  # noqa: F401

            trace = True
        except ImportError:
            os.environ["BASS_NEVER_TRACE"] = "1"
    res = run_bass_kernel_spmd(
        nc, in_maps, core_ids=list(range(N_CORES)), trace=trace
    )
    last_results = res
    y = np.stack(
        [np.asarray(res.results[i]["out"]).astype(np.float32).T for i in range(N_CORES)]
    )
    return np.ascontiguousarray(y)
